# revision 1
# baseline (speedup 1.0000x reference)
"""Dual-softmax cross-attention kernel for Trainium2 (Bass/Tile), 8 NeuronCores.

Problem: out = (0.3*softmax(q@kT) + 0.7*softmax(q2@kT)) @ v  projected by Wo + bo
  q  = x1 @ Wq, q2 = x2 @ Wq2, k = context @ Wk, v = context @ Wv
  shapes: x1/x2/context [4, 2048, 512]; 4 heads x 64 dim; out [4, 2048, 512].

Sharding: 8 cores = 4 batches x 2 query-halves. Each core computes the full
attention (all 4 heads) for its 1024 queries against the full 2048-key context
of its batch. No cross-core reductions needed; host concatenates outputs.

v2 design (trace-driven):
  - The attention phase is ACT(exp)-bound: 128 exp instructions of
    [128,2,512] at (1024+352)/1.2 ns = 143.5us/rep. Everything else is
    scheduled to hide under that stream.
  - q/k in bf16; the two heads of a pair are row-packed with
    tile_position=(0,0)/(64,0) so both score matmuls run concurrently
    (K=64 each) - halves scores PE time vs the zero-padded fp32r K=128 form.
  - Projections/transposes are emitted just-in-time as "units" injected one
    per jt-slot into the attention sweep, so ACT starts exp ~9us into the
    rep instead of after a ~90us serial prelude, and the PE never idles long
    enough to drop to the cold HAM clock.
  - U = [v | 1].T @ e (M=65 fused matmul) accumulated in PSUM gives both the
    un-normalized AV and the softmax denominator Z.
  - e/v/AV all bf16 (PSUM still accumulates fp32); every 4th key tile's exp
    runs on the DVE as a Schraudolph bit-trick tensor_scalar (int16 convert
    = bf16 exp bits), offloading ~25% of the exp stream from ACT.
  - jt pairs are processed scores+scores/exp+exp/AV+AV to halve the
    tiling-mode switches (row-tiled scores <-> full-array AV drains the PE).
  - 1/Z: the four Z rows of an (ic,p) block bounce through DRAM into a
    [128,16] layout where the exact iterative-divide reciprocal costs 8
    cycles per FREE element (vs 8*512 on a [1,512] row), then broadcast.
  - PSUM: "sc" tag [128,2,512] x3 bufs (6 banks) shared by scores, the
    transpose/projection scratch and the out-projection; "u" tag
    [65,2,512] x1 (2 banks) holds the per-sweep U accumulators.
  - Constant init (identity, vplus ones columns) hoisted out of the reps
    loop; pools are persistent so the For_i body has no pool-scope barriers.
"""

import numpy as np

import concourse.bacc as bacc
import concourse.mybir as mybir
import concourse.tile as tile
from concourse.masks import make_identity

F32 = mybir.dt.float32
BF16 = mybir.dt.bfloat16
F32R = mybir.dt.float32r

B, N, M_CTX = 4, 2048, 2048
C = 512        # query/context dim
H = 4          # heads
DH = 64        # dim per head
INNER = H * DH  # 256
E = 512        # output dim
SCALE = DH ** -0.5
N_CORES = 8
N_I = N // 2   # queries per core


def r(ap):
    """Bitcast an f32 AP to float32r for full-rate PE consumption."""
    return ap.bitcast(F32R)


def build_attention_nc(n_i=N_I, m=M_CTX, reps=1, variant="full", qk_bf16=True):
    n_ct = C // 128            # contraction tiles for the projections (4)
    n_jt = m // 128            # key tiles (16)
    ich = 512                  # query chunk (free dim of most matmuls)
    n_ic = n_i // ich          # 2
    n_g = m // 512             # ctx 512-row groups (4)
    DELAY = 3                  # AV matmuls trail the scores by this many jt

    nc = bacc.Bacc("TRN2", target_bir_lowering=False, debug=False,
                   num_devices=N_CORES)
    x1h = nc.declare_dram_parameter("x1h", [n_i, C], F32, isOutput=False)
    x2h = nc.declare_dram_parameter("x2h", [n_i, C], F32, isOutput=False)
    ctx = nc.declare_dram_parameter("ctx", [m, C], F32, isOutput=False)
    wq = nc.declare_dram_parameter("Wq", [C, INNER], F32, isOutput=False)
    wq2 = nc.declare_dram_parameter("Wq2", [C, INNER], F32, isOutput=False)
    wk = nc.declare_dram_parameter("Wk", [C, INNER], F32, isOutput=False)
    wv = nc.declare_dram_parameter("Wv", [C, INNER], F32, isOutput=False)
    wo = nc.declare_dram_parameter("Wo", [INNER, E], F32, isOutput=False)
    bo = nc.declare_dram_parameter("bo", [E], F32, isOutput=False)
    out = nc.declare_dram_parameter("out", [n_i, E], F32, isOutput=True)

    from contextlib import ExitStack
    with tile.TileContext(nc) as tc, ExitStack() as st:
        enter = st.enter_context
        consts = enter(tc.tile_pool(name="consts", bufs=1))
        persist = enter(tc.tile_pool(name="persist", bufs=1))
        xt_pool = enter(tc.tile_pool(name="xT", bufs=1))
        xnat_pool = enter(tc.tile_pool(name="xnat", bufs=8))
        ps = enter(tc.tile_pool(name="ps", bufs=1, space="PSUM"))
        e_pool = enter(tc.tile_pool(name="eT", bufs=8))
        usb_pool = enter(tc.tile_pool(name="usb", bufs=10))
        blend_pool = enter(tc.tile_pool(name="blend", bufs=4))
        o_pool = enter(tc.tile_pool(name="oT", bufs=6))
        osb_pool = enter(tc.tile_pool(name="osb", bufs=2))
        zdram_pool = enter(tc.tile_pool(name="zdram", bufs=4, space="DRAM"))

        # ---- one-time constant init (outside the reps loop) ----
        ident = consts.tile([128, 128], F32, tag="ident")
        make_identity(nc, ident)
        bo_bc = consts.tile([128, E], F32, tag="bo_bc")
        w_sb = {}
        for name in ("wq", "wq2", "wk", "wv"):
            for ct in range(n_ct):
                w_sb[name, ct] = consts.tile([128, INNER], F32,
                                             tag=f"{name}{ct}",
                                             name=f"{name}{ct}")
        wo_sb = [consts.tile([64, E], F32, tag=f"wo{h}", name=f"wo{h}")
                 for h in range(H)]

        # ---- persistent activations ----
        q1T = [persist.tile([128, n_i], BF16, tag=f"q1T{p}", name=f"q1T{p}")
               for p in range(2)]
        q2T = [persist.tile([128, n_i], BF16, tag=f"q2T{p}", name=f"q2T{p}")
               for p in range(2)]
        kT = [persist.tile([128, m], BF16, tag=f"kT{p}", name=f"kT{p}")
              for p in range(2)]
        vplus = [persist.tile([128, H, DH + 1], BF16, tag=f"vp{jt}",
                              name=f"vp{jt}")
                 for jt in range(n_jt)]
        for jt in range(n_jt):
            # ones columns for the Z row live at [:, h, DH]; the v part is
            # overwritten every rep, the ones persist.
            nc.vector.memset(vplus[jt][:], 1.0)

        x1T = [xt_pool.tile([128, n_i], F32, tag=f"x1T{ct}", name=f"x1T{ct}")
               for ct in range(n_ct)]
        x2T = [xt_pool.tile([128, n_i], F32, tag=f"x2T{ct}", name=f"x2T{ct}")
               for ct in range(n_ct)]
        cT = [xt_pool.tile([128, m], F32, tag=f"cT{ct}", name=f"cT{ct}")
              for ct in range(n_ct)]

        w_dram = {"wq": wq, "wq2": wq2, "wk": wk, "wv": wv}

        def emit_rep():
            # ================= DMA loads (sync queue, ordered) ===========
            def load_w(name):
                for ct in range(n_ct):
                    nc.sync.dma_start(
                        out=r(w_sb[name, ct][:]),
                        in_=r(w_dram[name][ct * 128:(ct + 1) * 128, :]))

            def load_nat(src, ig):
                nats = []
                for k in range(4):
                    t = xnat_pool.tile([128, C], F32, name="xnat")
                    nc.sync.dma_start(
                        out=t[:],
                        in_=src[(ig * 4 + k) * 128:(ig * 4 + k + 1) * 128, :])
                    nats.append(t)
                return nats

            # ================= A/B building blocks =======================
            def scratch():
                """One [128,512] PSUM scratch (half of a rotating sc tile)."""
                return ps.tile([128, 2, ich], F32, tag="sc", bufs=3, name="scratch")[:, 0, :]

            def emit_tr(nats, dstT, ct, ig, use_scalar=False):
                pt = scratch()
                for k in range(4):
                    nc.tensor.transpose(
                        pt[:, k * 128:(k + 1) * 128],
                        nats[k][:, ct * 128:(ct + 1) * 128],
                        ident[:])
                dst = r(dstT[ct][:, ig * 512:(ig + 1) * 512])
                if use_scalar:
                    nc.scalar.copy(dst, pt[:])
                else:
                    nc.vector.tensor_copy(dst, pt[:])

            def emit_kproj(p, g, use_scalar=False):
                pt = scratch()
                for ct in range(n_ct):
                    nc.tensor.matmul(
                        pt[:],
                        r(w_sb["wk", ct][:, p * 128:(p + 1) * 128]),
                        r(cT[ct][:, g * 512:(g + 1) * 512]),
                        start=(ct == 0), stop=(ct == n_ct - 1))
                dst = kT[p][:, g * 512:(g + 1) * 512]
                if use_scalar:
                    nc.scalar.copy(dst, pt[:])
                else:
                    nc.vector.tensor_copy(dst, pt[:])

            def emit_qproj(wname, dstT, p, ch, use_scalar=False):
                srcT = x1T if wname == "wq" else x2T
                pt = scratch()
                for ct in range(n_ct):
                    nc.tensor.matmul(
                        pt[:],
                        r(w_sb[wname, ct][:, p * 128:(p + 1) * 128]),
                        r(srcT[ct][:, ch * 512:(ch + 1) * 512]),
                        start=(ct == 0), stop=(ct == n_ct - 1))
                dst = dstT[p][:, ch * 512:(ch + 1) * 512]
                if use_scalar:
                    nc.scalar.copy(dst, pt[:])
                else:
                    nc.vector.tensor_copy(dst, pt[:])

            def emit_vproj(jt, use_scalar=False):
                pv = scratch()[:, 0:INNER]
                for ct in range(n_ct):
                    nc.tensor.matmul(
                        pv[:],
                        r(cT[ct][:, jt * 128:(jt + 1) * 128]),
                        r(w_sb["wv", ct][:]),
                        start=(ct == 0), stop=(ct == n_ct - 1))
                for h in range(H):
                    dst = vplus[jt][:, h, 0:DH]
                    src = pv[:, h * DH:(h + 1) * DH]
                    if use_scalar and h % 2 == 1:
                        nc.scalar.copy(dst, src)
                    else:
                        nc.vector.tensor_copy(dst, src)

            # ================= unit queue ================================
            units = []   # (fn, provides_key_or_None)
            done = set()

            def drain_one():
                if units:
                    fn, key = units.pop(0)
                    fn()
                    if key is not None:
                        done.add(key)

            def need(key):
                while key not in done:
                    assert units, f"dependency {key} not in unit queue"
                    drain_one()

            def queue_ctx_group(g):
                units.append((lambda g=g: pend.__setitem__(
                    ("ctx", g), load_nat(ctx, g)), None))
                for ct in range(n_ct):
                    units.append((lambda g=g, ct=ct: emit_tr(
                        pend[("ctx", g)], cT, ct, g), None))
                for p in range(2):
                    units.append((lambda p=p, g=g: emit_kproj(p, g), None))
                for j in range(4):
                    jt = g * 4 + j
                    key = ("ctx", g) if j == 3 else None
                    units.append((lambda jt=jt: emit_vproj(jt), key))

            def queue_x_group(which, ig):
                src, dstT, wname, qdst = (
                    (x1h, x1T, "wq", q1T) if which == 0 else
                    (x2h, x2T, "wq2", q2T))
                units.append((lambda: pend.__setitem__(
                    ("x", which, ig), load_nat(src, ig)), None))
                for ct in range(n_ct):
                    units.append((lambda ct=ct: emit_tr(
                        pend[("x", which, ig)], dstT, ct, ig), None))
                for p in range(2):
                    key = ("q", which, p, ig)
                    units.append((lambda p=p: emit_qproj(
                        wname, qdst, p, ig), key))

            pend = {}

            # ================= head: minimal serial prelude ==============
            pend[("ctx", 0)] = load_nat(ctx, 0)
            load_w("wk")
            load_w("wv")
            pend[("x", 0, 0)] = load_nat(x1h, 0)
            load_w("wq")
            load_w("wq2")
            nc.sync.dma_start(out=bo_bc[:], in_=bo.ap().partition_broadcast(128))
            for h in range(H):
                nc.sync.dma_start(out=r(wo_sb[h][:]),
                                  in_=r(wo[h * 64:(h + 1) * 64, :]))
            for ct in range(n_ct):
                emit_tr(pend[("ctx", 0)], cT, ct, 0, use_scalar=(ct % 2 == 1))
            for p in range(2):
                emit_kproj(p, 0, use_scalar=(p == 1))
            for jt in range(4):
                emit_vproj(jt, use_scalar=True)
            done.add(("ctx", 0))
            for ct in range(n_ct):
                emit_tr(pend[("x", 0, 0)], x1T, ct, 0, use_scalar=(ct % 2 == 0))
            for p in range(2):
                emit_qproj("wq", q1T, p, 0, use_scalar=(p == 0))
                done.add(("q", 0, p, 0))

            # remaining work, just-in-time injected into the sweeps
            for g in range(1, n_g):
                queue_ctx_group(g)
            queue_x_group(1, 0)       # x2 ig0 -> q2 ch0
            queue_x_group(0, 1)       # x1 ig1 -> q1 ch1
            queue_x_group(1, 1)       # x2 ig1 -> q2 ch1

            if variant == "dbgq":
                # drain all A/B units, then dump intermediates to `out`
                while units:
                    drain_one()
                row = [0]

                def dump(src_ap, width):
                    t = osb_pool.tile([128, E], F32, name="dump")
                    nc.vector.memset(t[:], 0.0)
                    nc.vector.tensor_copy(t[:, 0:width], src_ap)
                    nc.sync.dma_start(out=out[row[0]:row[0] + 128, :], in_=t[:])
                    row[0] += 128

                dump(q1T[0][:, 0:512], 512)
                dump(q1T[1][:, 0:512], 512)
                dump(q2T[0][:, 0:512], 512)
                dump(kT[0][:, 0:512], 512)
                dump(kT[1][:, 0:512], 512)
                dump(cT[0][:, 0:512], 512)
                dump(vplus[0][:].rearrange("p h d -> p (h d)"), H * (DH + 1))
                dump(x1T[0][:, 0:512], 512)
                return

            if variant == "dbgs":
                # one sweep (ic=0, s=0, p=0) with intermediate dumps
                row = [0]

                def dump2(src_ap, np_, width):
                    t = osb_pool.tile([128, E], F32, name="dump2")
                    nc.vector.memset(t[:], 0.0)
                    nc.vector.tensor_copy(t[0:np_, 0:width], src_ap)
                    nc.sync.dma_start(out=out[row[0]:row[0] + 128, :], in_=t[:])
                    row[0] += 128

                need(("q", 0, 0, 0))
                u_ps = ps.tile([DH + 1, 2, ich], F32, tag="u",
                               bufs=1, name="u_ps")
                ets = {}
                for step in range(n_jt + DELAY):
                    if step < n_jt:
                        jt = step
                        need(("ctx", jt // 4))
                        jsl = slice(jt * 128, (jt + 1) * 128)
                        sc = ps.tile([128, 2, ich], F32, tag="sc",
                                     bufs=3, name="sc")
                        for h2 in range(2):
                            psl = slice(h2 * 64, (h2 + 1) * 64)
                            nc.tensor.matmul(
                                sc[:, h2, :], kT[0][psl, jsl],
                                q1T[0][psl, 0:ich],
                                start=True, stop=True,
                                tile_position=(h2 * 64, 0))
                        et = e_pool.tile([128, 2, ich], F32, name="et")
                        nc.scalar.activation(
                            r(et[:]), sc[:],
                            mybir.ActivationFunctionType.Exp,
                            scale=SCALE)
                        ets[jt] = et
                        if jt == 0:
                            dump2(et[:, 0, :], 128, 512)
                            dump2(et[:, 1, :], 128, 512)
                    if step >= DELAY:
                        jt = step - DELAY
                        et = ets.pop(jt)
                        for h2 in range(2):
                            nc.tensor.matmul(
                                u_ps[:, h2, :],
                                r(vplus[jt][:, h2, :]),
                                r(et[:, h2, :]),
                                start=(jt == 0), stop=(jt == n_jt - 1))
                for h2 in range(2):
                    ut = usb_pool.tile([DH + 1, ich], F32, name="ut")
                    nc.vector.tensor_copy(ut[:], u_ps[:, h2, :])
                    dump2(ut[:], DH + 1, 512)
                    # blend-side pieces: recip Z + broadcast
                    nc.vector.reciprocal_approx_fast(
                        ut[DH:DH + 1, :], ut[DH:DH + 1, :])
                    zd = zdram_pool.tile([1, ich], F32, tag="zd", name="zd")
                    nc.sync.dma_start(out=zd[:], in_=ut[DH:DH + 1, :])
                    rb = blend_pool.tile([64, ich], F32, tag="rb", name="rb")
                    nc.sync.dma_start(out=rb[:],
                                      in_=zd[:].partition_broadcast(64))
                    dump2(rb[:], 64, 512)
                while units:
                    drain_one()
                return

            # ================= attention sweeps ==========================
            u_store = {}
            oT_store = {}

            def emit_blend(ic, p):
                # Gather the 4 Z rows into DRAM, reload as [128,16] so the
                # exact iterative-divide reciprocal (8 cyc per FREE element)
                # costs ~130 cycles instead of 8*512 per [1,512] row, then
                # store back and partition-broadcast each row.
                zd4 = zdram_pool.tile([4, ich], F32, tag="zd4", name="zd4")
                for s in range(2):
                    for h2 in range(2):
                        u_sb = u_store[(ic, p, s, h2)]
                        idx = 2 * s + h2
                        nc.sync.dma_start(out=zd4[idx:idx + 1, :],
                                          in_=u_sb[DH:DH + 1, :])
                zt = blend_pool.tile([128, 16], F32, tag="zt", name="zt")
                zview = zd4[:].rearrange("a (c f) -> (a c) f", c=32)
                nc.sync.dma_start(out=zt[:], in_=zview)
                nc.vector.reciprocal(zt[:], zt[:])
                nc.sync.dma_start(out=zview, in_=zt[:])
                for h2 in range(2):
                    oh = o_pool.tile([64, ich], F32, tag="oh", name="oh")
                    tmp = blend_pool.tile([64, ich], F32, tag="bt", name="bt")
                    for s, coef in ((0, 0.3), (1, 0.7)):
                        u_sb = u_store.pop((ic, p, s, h2))
                        idx = 2 * s + h2
                        rb = blend_pool.tile([64, ich], F32, tag="rb", name="rb")
                        nc.sync.dma_start(
                            out=rb[:],
                            in_=zd4[idx:idx + 1, :].partition_broadcast(64))
                        dst = tmp[:] if s == 0 else r(oh[:])
                        nc.vector.scalar_tensor_tensor(
                            dst, u_sb[0:DH, :], coef, rb[:],
                            op0=mybir.AluOpType.mult,
                            op1=mybir.AluOpType.mult)
                    nc.vector.tensor_add(r(oh[:]), oh[:], tmp[:])
                    oT_store[ic, 2 * p + h2] = oh

            def emit_outproj(ic):
                oT = [oT_store.pop((ic, h)) for h in range(H)]
                for mt in range(ich // 128):
                    po = ps.tile([128, 2, ich], F32, tag="sc", bufs=3, name="po")[:, 0, :]
                    for h in range(H):
                        nc.tensor.matmul(
                            po[:],
                            r(oT[h][:, mt * 128:(mt + 1) * 128]),
                            r(wo_sb[h][:]),
                            start=(h == 0), stop=(h == H - 1))
                    ob = osb_pool.tile([128, E], F32, name="ob")
                    nc.vector.tensor_add(ob[:], po[:], bo_bc[:])
                    nc.sync.dma_start(
                        out=out[ic * ich + mt * 128:ic * ich + (mt + 1) * 128, :],
                        in_=ob[:])

            # Schraudolph fast-exp constants: exp(SCALE*x) ~
            # bitcast_f32(int32(A*x + B)); ~3% max rel err, applied to every
            # 4th key tile to offload ~25% of the exp stream from ACT to DVE.
            SCH_A = float(SCALE * 1.4426950408889634 * 8388608.0 / 65536.0)
            SCH_B = float((127 * 8388608 - 366000) / 65536.0)
            I16 = mybir.dt.int16

            for ic in range(n_ic):
                isl = slice(ic * ich, (ic + 1) * ich)
                order = ([(0, 0), (0, 1), (1, 0), (1, 1)] if ic == 0 else
                         [(0, 0), (1, 0), (0, 1), (1, 1)])
                for s, p in order:
                    qT = q1T if s == 0 else q2T
                    if True:
                        need(("q", s, p, ic))
                        u_ps = ps.tile([DH + 1, 2, ich], F32, tag="u",
                                       bufs=1, name="u_ps")
                        ets = {}
                        # jt pairs: both scores pairs, then both exps, then
                        # the previous pair's AVs - batching keeps the PE in
                        # one tiling mode longer (mode switches drain the
                        # array) and shortens the LDW/MM interleave.
                        for step in range(0, n_jt + 2, 2):
                            if step < n_jt:
                                for jt in (step, step + 1):
                                    need(("ctx", jt // 4))
                                    jsl = slice(jt * 128, (jt + 1) * 128)
                                    sc = ps.tile([128, 2, ich], F32, tag="sc",
                                                 bufs=3, name="sc")
                                    for h2 in range(2):
                                        psl = slice(h2 * 64, (h2 + 1) * 64)
                                        nc.tensor.matmul(
                                            sc[:, h2, :], kT[p][psl, jsl],
                                            qT[p][psl, isl],
                                            start=True, stop=True,
                                            tile_position=(h2 * 64, 0))
                                    et = e_pool.tile([128, 2, ich], BF16,
                                                     name="et")
                                    if jt % 4 == 1:
                                        nc.vector.tensor_scalar(
                                            et[:].bitcast(I16), sc[:],
                                            SCH_A, SCH_B,
                                            op0=mybir.AluOpType.mult,
                                            op1=mybir.AluOpType.add)
                                    else:
                                        nc.scalar.activation(
                                            et[:], sc[:],
                                            mybir.ActivationFunctionType.Exp,
                                            scale=SCALE)
                                    ets[jt] = et
                                if step % 4 == 0:
                                    drain_one()
                            if step >= 2:
                                for jt in (step - 2, step - 1):
                                    et = ets.pop(jt)
                                    for h2 in range(2):
                                        nc.tensor.matmul(
                                            u_ps[:, h2, :],
                                            vplus[jt][:, 2 * p + h2, :],
                                            et[:, h2, :],
                                            start=(jt == 0),
                                            stop=(jt == n_jt - 1))
                        for h2 in range(2):
                            ut = usb_pool.tile([DH + 1, ich], F32, name="ut")
                            nc.vector.tensor_copy(ut[:], u_ps[:, h2, :])
                            u_store[ic, p, s, h2] = ut
                        if s == 1:
                            units.append(
                                (lambda ic=ic, p=p: emit_blend(ic, p), None))
                            if (s, p) == order[3]:
                                units.append(
                                    (lambda ic=ic: emit_outproj(ic), None))
            while units:
                drain_one()

        if reps == 1:
            emit_rep()
        elif reps % 2 == 0:
            # two reps per For_i body: the loop's all-engine reset barrier
            # fires half as often, and rep B's DMA head + transposes overlap
            # rep A's blend/out-projection tail (PE queue stays contiguous,
            # so the HAM clock stays warm across the seam).
            with tc.For_i(0, reps // 2, 1):
                emit_rep()
                emit_rep()
        else:
            with tc.For_i(0, reps, 1):
                emit_rep()

    nc.compile()
    return nc


_NC_CACHE = {}


def _get_nc():
    if "nc" not in _NC_CACHE:
        _NC_CACHE["nc"] = build_attention_nc()
    return _NC_CACHE["nc"]


def kernel(x1, x2, context, Wq, Wq2, Wk, Wv, Wo, bo):
    from concourse.bass_utils import run_bass_kernel_spmd

    nc = _get_nc()
    x1 = np.ascontiguousarray(np.asarray(x1, dtype=np.float32))
    x2 = np.ascontiguousarray(np.asarray(x2, dtype=np.float32))
    context = np.ascontiguousarray(np.asarray(context, dtype=np.float32))
    shared = {
        "Wq": np.ascontiguousarray(np.asarray(Wq, np.float32)),
        "Wq2": np.ascontiguousarray(np.asarray(Wq2, np.float32)),
        "Wk": np.ascontiguousarray(np.asarray(Wk, np.float32)),
        "Wv": np.ascontiguousarray(np.asarray(Wv, np.float32)),
        "Wo": np.ascontiguousarray(np.asarray(Wo, np.float32)),
        "bo": np.ascontiguousarray(np.asarray(bo, np.float32)),
    }
    in_maps = []
    for core in range(N_CORES):
        b, half = divmod(core, 2)
        qsl = slice(half * N_I, (half + 1) * N_I)
        in_maps.append({
            "x1h": np.ascontiguousarray(x1[b, qsl]),
            "x2h": np.ascontiguousarray(x2[b, qsl]),
            "ctx": np.ascontiguousarray(context[b]),
            **shared,
        })
    res = run_bass_kernel_spmd(nc, in_maps, core_ids=list(range(N_CORES)))
    full = np.empty((B, N, E), dtype=np.float32)
    for core in range(N_CORES):
        b, half = divmod(core, 2)
        full[b, half * N_I:(half + 1) * N_I] = res.results[core]["out"]
    return full

